# revision 17
# baseline (speedup 1.0000x reference)
"""GAT(2-layer, 8 heads) + MLP — full on-device Bass SPMD implementation.

Design (8 NeuronCores, graph/data parallel):
  - x is replicated (host sends xT bf16); every core computes the FULL h1
    node-transform table (redundant PE work is cheaper than exchanging it),
    writing rows [h1(256) | alpha_src(8)] bf16 into two DRAM half-tables
    (dma_gather indices are int16, so tables are limited to <32768 rows).
  - Edge phase 1 is dst-sharded: core c owns nodes [c*SH, (c+1)*SH). Edges
    (sorted by dst group, split by src half, padded to 128-edge tiles) are
    gathered with batched dma_gather; attention softmax (no max-subtraction;
    exp args are O(4)) and weighted aggregation run via per-tile
    is_equal-selection matmuls accumulated in PSUM.
  - out1^T (bf16) is AllGathered; every core then computes the full h2 table
    (layer 2), runs its edge phase 2, head-mean, and the tiny MLP, emitting
    outT [2, SH] f32 per core.
  - alpha_dst for own nodes is read from an extract-all SBUF buffer with a
    partition_id()-based dynamic slice (keeps the program SPMD-uniform).

Numerics: bf16 tables/matmuls, fp32 PSUM + softmax intermediates.
"""

import numpy as np

IN_CH = 128
HEADS = 8
HID = 32
GOUT = 64
MLP_HID = 64
OUT_CLS = 2
NEG_SLOPE = 0.2
NCORES = 8

D1H = HEADS * HID          # 256
D2H = HEADS * GOUT         # 512
DC1 = 384                  # L1 table row cols (768B)
DC2 = 640                  # L2 table row cols (1280B)


def plan_graph(src_pid, dst_shard, dst_local_in_shard, n_real_shard, sh, np_pad):
    """Build the shared tile structure + per-core edge arrays.

    Returns (plan, per_core) where plan holds compile-time lists and
    per_core holds the idx16/dstl input arrays.
    """
    groups = sh // 128
    half_rows = np_pad // 2
    counts = np.zeros((NCORES, groups, 2), np.int64)
    core_data = []
    for c in range(NCORES):
        sel = dst_shard == c
        dl = dst_local_in_shard[sel]
        sp = src_pid[sel]
        half = (sp >= half_rows).astype(np.int64)
        sloc = sp - half * half_rows
        grp = dl >> 7
        dstl = dl & 127
        order = np.lexsort((half, grp))
        grp, half, sloc, dstl = grp[order], half[order], sloc[order], dstl[order]
        key = grp * 2 + half
        counts[c] = np.bincount(key, minlength=groups * 2).reshape(groups, 2)
        core_data.append((key, sloc, dstl))

    tiles = np.ceil(counts.max(axis=0) / 128.0).astype(np.int64)  # [groups, 2]

    # stream order: per quad of 4 groups, all A tiles then all B tiles
    quads = [list(range(q, min(q + 4, groups))) for q in range(0, groups, 4)]
    tile_list = []          # (g, h) per tile
    chunks = []             # (half, start_tile, ntiles) in stream order
    quad_chunks = []        # per quad: list of chunks
    group_tiles = [[None, None] for _ in range(groups)]  # (start_tile, n) per half
    for quad in quads:
        qc = []
        for h in (0, 1):
            start = len(tile_list)
            for g in quad:
                group_tiles[g][h] = (len(tile_list), int(tiles[g][h]))
                tile_list += [(g, h)] * int(tiles[g][h])
            if len(tile_list) > start:
                chunks.append((h, start, len(tile_list) - start))
                qc.append((h, start, len(tile_list) - start))
        quad_chunks.append(qc)
    nt = len(tile_list)

    first_tile = np.zeros((groups, 2), np.int64)
    for g in range(groups):
        for h in (0, 1):
            first_tile[g, h] = group_tiles[g][h][0]

    per_core = []
    for c in range(NCORES):
        key, sloc, dstl = core_data[c]
        # rank within (g,h) segment
        seg_start = np.zeros(groups * 2, np.int64)
        cnt = counts[c].reshape(-1)
        np.cumsum(cnt[:-1], out=seg_start[1:])
        rank = np.arange(len(key)) - seg_start[key]
        slot = first_tile.reshape(-1)[key] * 128 + rank
        idx_flat = np.zeros(nt * 128, np.int16)
        dstl_flat = np.full(nt * 128, -1.0, np.float32)
        idx_flat[slot] = sloc.astype(np.int16)
        dstl_flat[slot] = dstl.astype(np.float32)
        idx16 = np.tile(idx_flat.reshape(-1, 16).T, (8, 1))  # [128, nt*8]
        dstl_in = dstl_flat.reshape(nt, 128).T.copy()        # [128, nt]
        per_core.append((idx16, dstl_in))

    plan = dict(groups=groups, quads=quads, chunks=chunks,
                group_tiles=group_tiles, nt=nt, half_rows=half_rows,
                tile_list=tile_list, quad_chunks=quad_chunks)
    return plan, per_core


def build_program(n_real, plan, use_prelu=True):
    import sys
    for p in ("/opt/trn_rl_repo",):
        if p not in sys.path:
            sys.path.append(p)
    from concourse import bacc, mybir, bass
    import concourse.tile as tile
    from concourse.masks import make_identity
    from concourse.bass import ds, ts
    import concourse.bass as _B
    import inspect as _inspect
    import textwrap as _tw

    # dma_gather with the 256B elem_size restriction lifted (transpose-only
    # constraint; non-transpose path handles arbitrary elem sizes).
    _src = _inspect.getsource(_B.BassGpSimd.dma_gather)
    _src = _src.replace(
        """        assert (
            elem_size_bytes > 0 and elem_size_bytes % 256 == 0
        )  # transpose restriction""", "        assert elem_size_bytes > 0")
    _ns = dict(_B.__dict__)
    exec(_tw.dedent(_src), _ns)
    dma_gather_raw = _ns["dma_gather"]

    f32 = mybir.dt.float32
    bf16 = mybir.dt.bfloat16
    i16 = mybir.dt.int16
    AF = mybir.ActivationFunctionType

    groups = plan["groups"]
    sh = groups * 128
    np_pad = NCORES * sh
    half_rows = plan["half_rows"]
    half_tiles = half_rows // 128
    n_tiles = np_pad // 128
    nt = plan["nt"]
    own_stride = groups * HEADS  # per-core alpha_d columns

    nc = bacc.Bacc("TRN2", target_bir_lowering=False, debug=False,
                   num_devices=NCORES, num_swdge_queues=4)

    # ---- I/O ----
    xt_in = nc.dram_tensor("xt", [IN_CH, sh], bf16, kind="ExternalInput")
    idx_in = nc.dram_tensor("idx16", [128, nt * 8], i16, kind="ExternalInput")
    dstl_in = nc.dram_tensor("dstl", [128, nt], f32, kind="ExternalInput")
    iota_in = nc.dram_tensor("iotarow", [128, 128], f32, kind="ExternalInput")
    w1_in = nc.dram_tensor("w1aug", [IN_CH, 272], bf16, kind="ExternalInput")
    w2a_in = nc.dram_tensor("w2k0", [128, 528], bf16, kind="ExternalInput")
    w2b_in = nc.dram_tensor("w2k1", [128, 528], bf16, kind="ExternalInput")
    wm1_in = nc.dram_tensor("wm1", [MLP_HID, MLP_HID], bf16, kind="ExternalInput")
    wm2_in = nc.dram_tensor("wm2", [MLP_HID, OUT_CLS], bf16, kind="ExternalInput")
    b1_in = nc.dram_tensor("b1res", [128, D1H], f32, kind="ExternalInput")
    b2_in = nc.dram_tensor("b2x8", [128, GOUT], f32, kind="ExternalInput")
    bm1_in = nc.dram_tensor("bm1col", [MLP_HID, 1], f32, kind="ExternalInput")
    bm2_in = nc.dram_tensor("bm2col", [OUT_CLS, 1], f32, kind="ExternalInput")
    out_t = nc.dram_tensor("outT", [OUT_CLS, sh], f32, kind="ExternalOutput")

    # ---- internal DRAM ----
    agin1 = nc.dram_tensor("agin1", [sh, DC1], bf16)
    agbuf1 = nc.dram_tensor("agbuf1", [NCORES * sh, DC1], bf16,
                            addr_space="Shared")
    agin2 = nc.dram_tensor("agin2", [sh, DC2], bf16)
    agbuf2 = nc.dram_tensor("agbuf2", [NCORES * sh, DC2], bf16,
                            addr_space="Shared")

    with tile.TileContext(nc) as tc:
        with tc.tile_pool(name="res", bufs=1) as res:
            # residents
            iota = res.tile([128, 128], f32, tag="iota")
            nc.sync.dma_start(iota[:], iota_in[:])
            ident = res.tile([128, 128], bf16, tag="ident")
            make_identity(nc, ident[:])
            idxt = res.tile([128, nt * 8], i16, tag="idx")
            nc.sync.dma_start(idxt[:], idx_in[:])
            dstl = res.tile([128, nt], f32, tag="dstl")
            nc.sync.dma_start(dstl[:], dstl_in[:])
            b1r = res.tile([128, D1H], f32, tag="b1r")
            nc.sync.dma_start(b1r[:], b1_in[:])
            b2r = res.tile([128, GOUT], f32, tag="b2r")
            nc.sync.dma_start(b2r[:], b2_in[:])
            wm1 = res.tile([MLP_HID, MLP_HID], bf16, tag="wm1")
            nc.sync.dma_start(wm1[:], wm1_in[:])
            wm2 = res.tile([MLP_HID, OUT_CLS], bf16, tag="wm2")
            nc.sync.dma_start(wm2[:], wm2_in[:])
            bm1 = res.tile([MLP_HID, 1], f32, tag="bm1")
            nc.sync.dma_start(bm1[:], bm1_in[:])
            bm2 = res.tile([OUT_CLS, 1], f32, tag="bm2")
            nc.sync.dma_start(bm2[:], bm2_in[:])
            adx = res.tile([128, groups * HEADS], bf16, tag="adx")
            adx2 = res.tile([128, groups * HEADS], bf16, tag="adx2")

            # ---- generic edge phase ----
            def edge_phase(tbl_a, tbl_b, agin, dcols, dh, finalize, ad_ap, gpool,
                           wpool, ps_num, ps_small, ps_t):
                tile_group = plan["tile_list"]
                oct_art = {}
                qn = 0
                for qi, quad in enumerate(plan["quads"]):
                    for (h, cstart, ntl) in plan["quad_chunks"][qi]:
                        tbl = tbl_b if h else tbl_a
                        for k0 in range(0, ntl, 8):
                            kn = min(8, ntl - k0)
                            gt0 = cstart + k0
                            el = dh + 8
                            g_t = gpool.tile([128, 8, el], bf16, tag="g")
                            dma_gather_raw(
                                nc.gpsimd,
                                out_ap=g_t[:, 0:kn, :], in_ap=tbl,
                                idxs_ap=idxt[:, gt0 * 8:(gt0 + kn) * 8],
                                num_idxs=kn * 128, num_idxs_reg=kn * 128,
                                elem_size=el, elem_step=dcols, queue_num=qn % 4)
                            qn += 1
                            sal = wpool.tile([128, 8, 128], bf16, tag="S")
                            nc.vector.tensor_tensor(
                                out=sal[:, 0:kn, :],
                                in0=dstl[:, gt0:gt0 + kn].unsqueeze(2).to_broadcast(
                                    [128, kn, 128]),
                                in1=iota[:].unsqueeze(1).to_broadcast([128, kn, 128]),
                                op=mybir.AluOpType.is_equal)
                            pad_all = ps_small.tile([128, 64], f32, tag="pad")
                            for j in range(kn):
                                grp = tile_group[gt0 + j][0]
                                pst = ps_t.tile([128, 128], bf16, tag="pst")
                                nc.tensor.transpose(pst[:], sal[:, j, :], ident[:])
                                st = wpool.tile([128, 128], bf16, tag="st")
                                nc.any.tensor_copy(st[:], pst[:])
                                nc.tensor.matmul(
                                    pad_all[:, j * 8:(j + 1) * 8], lhsT=st[:],
                                    rhs=ad_ap(grp), start=True, stop=True)
                            esum = wpool.tile([128, 64], f32, tag="esum")
                            nc.vector.tensor_tensor(
                                out=esum[:, 0:kn * 8].rearrange(
                                    "p (a b) -> p a b", a=kn),
                                in0=g_t[:, 0:kn, dh:dh + 8],
                                in1=pad_all[:, 0:kn * 8].rearrange(
                                    "p (a b) -> p a b", a=kn),
                                op=mybir.AluOpType.add)
                            elr = wpool.tile([128, 64], f32, tag="elr")
                            if use_prelu:
                                nc.scalar.activation(elr[:, 0:kn * 8],
                                                     esum[:, 0:kn * 8], AF.Prelu,
                                                     alpha=NEG_SLOPE)
                            else:
                                esc = wpool.tile([128, 64], f32, tag="esc")
                                nc.vector.tensor_scalar_mul(
                                    esc[:, 0:kn * 8], esum[:, 0:kn * 8], NEG_SLOPE)
                                nc.vector.tensor_tensor(
                                    out=elr[:, 0:kn * 8], in0=esum[:, 0:kn * 8],
                                    in1=esc[:, 0:kn * 8], op=mybir.AluOpType.max)
                            ex = wpool.tile([128, 64], bf16, tag="ex")
                            nc.scalar.activation(ex[:, 0:kn * 8], elr[:, 0:kn * 8],
                                                 AF.Exp)
                            ch = dh // HEADS
                            wg = wpool.tile([128, 8, dh], bf16, tag="wg")
                            nc.vector.tensor_tensor(
                                out=wg[:, 0:kn, :].rearrange(
                                    "p a (h c) -> p a h c", h=HEADS),
                                in0=g_t[:, 0:kn, 0:dh].rearrange(
                                    "p a (h c) -> p a h c", h=HEADS),
                                in1=ex[:, 0:kn * 8].rearrange(
                                    "p (a h) -> p a h", a=kn).unsqueeze(3)
                                    .to_broadcast([128, kn, HEADS, ch]),
                                op=mybir.AluOpType.mult)
                            for j in range(kn):
                                oct_art[gt0 + j] = (sal, ex, wg, j)
                    for g in quad:
                        pnum = ps_num.tile([128, dh], f32, tag="num")
                        pden = ps_small.tile([128, HEADS], f32, tag="den")
                        # self-loop tile: direct local read from agin (own shard)
                        gs = wpool.tile([128, dh + 8], bf16, tag="gs")
                        nc.sync.dma_start(gs[:], agin[g * 128:(g + 1) * 128, 0:dh + 8])
                        esums = wpool.tile([128, HEADS], f32, tag="esums")
                        nc.vector.tensor_tensor(out=esums[:], in0=gs[:, dh:dh + 8],
                                                in1=ad_ap(g), op=mybir.AluOpType.add)
                        elrs = wpool.tile([128, HEADS], f32, tag="elrs")
                        if use_prelu:
                            nc.scalar.activation(elrs[:], esums[:], AF.Prelu,
                                                 alpha=NEG_SLOPE)
                        else:
                            escs = wpool.tile([128, HEADS], f32, tag="escs")
                            nc.vector.tensor_scalar_mul(escs[:], esums[:], NEG_SLOPE)
                            nc.vector.tensor_tensor(out=elrs[:], in0=esums[:],
                                                    in1=escs[:],
                                                    op=mybir.AluOpType.max)
                        exs = wpool.tile([128, HEADS], bf16, tag="exs")
                        nc.scalar.activation(exs[:], elrs[:], AF.Exp)
                        ch = dh // HEADS
                        wgs = wpool.tile([128, dh], bf16, tag="wgs")
                        nc.vector.tensor_tensor(
                            out=wgs[:].rearrange("p (h c) -> p h c", h=HEADS),
                            in0=gs[:, 0:dh].rearrange("p (h c) -> p h c", h=HEADS),
                            in1=exs[:].unsqueeze(2).to_broadcast([128, HEADS, ch]),
                            op=mybir.AluOpType.mult)
                        tl = []
                        for h in (0, 1):
                            st_t, n_t = plan["group_tiles"][g][h]
                            tl += [st_t + i for i in range(n_t)]
                        nc.tensor.matmul(pnum[:], lhsT=ident[:], rhs=wgs[:],
                                         start=True, stop=(len(tl) == 0))
                        nc.tensor.matmul(pden[:], lhsT=ident[:], rhs=exs[:],
                                         start=True, stop=(len(tl) == 0))
                        for j, t in enumerate(tl):
                            sal, ex, wg, jj = oct_art.pop(t)
                            nc.tensor.matmul(pnum[:], lhsT=sal[:, jj, :],
                                             rhs=wg[:, jj, :],
                                             start=False, stop=(j == len(tl) - 1))
                            nc.tensor.matmul(pden[:], lhsT=sal[:, jj, :],
                                             rhs=ex[:, jj * 8:(jj + 1) * 8],
                                             start=False, stop=(j == len(tl) - 1))
                        finalize(g, pnum, pden, ps_t=ps_t, ps_num=ps_num,
                                 ps_small=ps_small)

            # ---- phase 1: mm1' (own shard) -> agin1; AG1 -> full L1 table ----
            with tc.tile_pool(name="mm1", bufs=1) as mm1p, \
                 tc.tile_pool(name="stg1", bufs=3) as stg1, \
                 tc.tile_pool(name="ps_mm1", bufs=3, space="PSUM") as ps_mm1:
                xt = mm1p.tile([IN_CH, sh], bf16, tag="xt")
                nc.sync.dma_start(xt[:], xt_in[:])
                w1 = mm1p.tile([IN_CH, 272], bf16, tag="w1")
                nc.sync.dma_start(w1[:], w1_in[:])
                bat = 4
                for t0 in range(0, groups, bat):
                    nb = min(bat, groups - t0)
                    stg = stg1.tile([128, bat, 264], bf16, tag="stg")
                    for j in range(nb):
                        t = t0 + j
                        pm = ps_mm1.tile([128, 272], f32, tag="pmm")
                        nc.tensor.matmul(pm[:], lhsT=xt[:, t * 128:(t + 1) * 128],
                                         rhs=w1[:], start=True, stop=True)
                        nc.any.tensor_copy(stg[:, j, 0:264], pm[:, 0:264])
                        nc.any.tensor_copy(adx[:, t * 8:(t + 1) * 8], pm[:, 264:272])
                    dst_ap = agin1[t0 * 128:(t0 + nb) * 128, 0:264].rearrange(
                        "(b r) c -> r b c", b=nb)
                    nc.sync.dma_start(dst_ap, stg[:, 0:nb, :])

            nc.gpsimd.collective_compute(
                "AllGather", mybir.AluOpType.bypass,
                replica_groups=[list(range(NCORES))],
                ins=[agin1[:]], outs=[agbuf1[:]])

            # ---- phase 2: edge phase 1 -> o1sb (transposed, SBUF) ----
            with tc.tile_pool(name="o1p", bufs=1) as o1p:
                o1sb = o1p.tile([128, 2, sh], bf16, tag="o1sb")

                def fin1(g, pnum, pden, ps_t=None, ps_num=None, ps_small=None):
                    den = nc_pool.tile([128, HEADS], f32, tag="denc")
                    nc.any.tensor_copy(den[:], pden[:])
                    rec = nc_pool.tile([128, HEADS], f32, tag="rec")
                    nc.vector.reciprocal(rec[:], den[:])
                    v = nc_pool.tile([128, D1H], f32, tag="v1")
                    nc.vector.tensor_tensor(
                        out=v[:].rearrange("p (h c) -> p h c", h=HEADS),
                        in0=pnum[:].rearrange("p (h c) -> p h c", h=HEADS),
                        in1=rec[:].unsqueeze(2).to_broadcast([128, HEADS, HID]),
                        op=mybir.AluOpType.mult)
                    va = nc_pool.tile([128, D1H], f32, tag="va1")
                    nc.vector.tensor_tensor(out=va[:], in0=v[:], in1=b1r[:],
                                            op=mybir.AluOpType.add)
                    h1r = nc_pool.tile([128, D1H], bf16, tag="h1r")
                    nc.scalar.activation(h1r[:], va[:], AF.Relu)
                    for i in range(2):
                        pt = ps_t.tile([128, 128], bf16, tag="pst")
                        nc.tensor.transpose(pt[:], h1r[:, i * 128:(i + 1) * 128],
                                            ident[:])
                        nc.any.tensor_copy(o1sb[:, i, g * 128:(g + 1) * 128], pt[:])

                with tc.tile_pool(name="e1", bufs=8) as gpool1, \
                     tc.tile_pool(name="e1w", bufs=4) as nc_pool, \
                     tc.tile_pool(name="ps1n", bufs=2, space="PSUM") as ps_num, \
                     tc.tile_pool(name="ps1s", bufs=2, space="PSUM") as ps_small, \
                     tc.tile_pool(name="ps1t", bufs=2, space="PSUM") as ps_t:
                    edge_phase(agbuf1[0:half_rows, 0:D1H + 8],
                               agbuf1[half_rows:NCORES * sh, 0:D1H + 8],
                               agin1, DC1, D1H, fin1,
                               lambda grp: adx[:, grp * 8:(grp + 1) * 8],
                               gpool1, nc_pool, ps_num, ps_small, ps_t)

                # ---- phase 3: mm2' (own shard only) -> agin2 ----
                with tc.tile_pool(name="mm2", bufs=1) as mm2p, \
                     tc.tile_pool(name="stg2", bufs=3) as stg2, \
                     tc.tile_pool(name="ps_mm2", bufs=2, space="PSUM") as ps_mm2:
                    w2sb = []
                    for kb in range(2):
                        w2t = mm2p.tile([128, 528], bf16, tag=f"w2_{kb}")
                        w2sb.append(w2t)
                    nc.sync.dma_start(w2sb[0][:], w2a_in[:])
                    nc.sync.dma_start(w2sb[1][:], w2b_in[:])
                    bat = 4
                    for lt0 in range(0, groups, bat):
                        nb = min(bat, groups - lt0)
                        stg = stg2.tile([128, bat, 520], bf16, tag="stg")
                        for j in range(nb):
                            lt = lt0 + j
                            pm = ps_mm2.tile([128, 512], f32, tag="num")
                            pmb = ps_mm2.tile([128, 16], f32, tag="pad")
                            for kb in range(2):
                                nc.tensor.matmul(
                                    pm[:], lhsT=o1sb[:, kb, lt * 128:(lt + 1) * 128],
                                    rhs=w2sb[kb][:, 0:512],
                                    start=(kb == 0), stop=(kb == 1))
                            for kb in range(2):
                                nc.tensor.matmul(
                                    pmb[:], lhsT=o1sb[:, kb, lt * 128:(lt + 1) * 128],
                                    rhs=w2sb[kb][:, 512:528],
                                    start=(kb == 0), stop=(kb == 1))
                            nc.any.tensor_copy(stg[:, j, 0:512], pm[:])
                            nc.any.tensor_copy(stg[:, j, 512:520], pmb[:, 0:8])
                            nc.any.tensor_copy(adx2[:, lt * 8:(lt + 1) * 8],
                                               pmb[:, 8:16])
                        dst_ap = agin2[lt0 * 128:(lt0 + nb) * 128, 0:520].rearrange(
                            "(b r) c -> r b c", b=nb)
                        nc.sync.dma_start(dst_ap, stg[:, 0:nb, :])

            # ---- phase 4: AllGather shard tables -> full L2 gather table ----
            nc.gpsimd.collective_compute(
                "AllGather", mybir.AluOpType.bypass,
                replica_groups=[list(range(NCORES))],
                ins=[agin2[:]], outs=[agbuf2[:]])

            # ---- phase 5: edge phase 2 + MLP ----
            def fin2(g, pnum, pden, ps_t=None, ps_num=None, ps_small=None):
                den = nc_pool.tile([128, HEADS], f32, tag="denc")
                nc.any.tensor_copy(den[:], pden[:])
                rec = nc_pool.tile([128, HEADS], f32, tag="rec")
                nc.vector.reciprocal(rec[:], den[:])
                v = nc_pool.tile([128, D2H], f32, tag="v2")
                nc.vector.tensor_tensor(
                    out=v[:].rearrange("p (h c) -> p h c", h=HEADS),
                    in0=pnum[:].rearrange("p (h c) -> p h c", h=HEADS),
                    in1=rec[:].unsqueeze(2).to_broadcast([128, HEADS, GOUT]),
                    op=mybir.AluOpType.mult)
                m1 = nc_pool.tile([128, 256], f32, tag="m1")
                nc.vector.tensor_tensor(out=m1[:], in0=v[:, 0:256], in1=v[:, 256:512],
                                        op=mybir.AluOpType.add)
                m2 = nc_pool.tile([128, 128], f32, tag="m2")
                nc.vector.tensor_tensor(out=m2[:], in0=m1[:, 0:128], in1=m1[:, 128:256],
                                        op=mybir.AluOpType.add)
                m3 = nc_pool.tile([128, GOUT], f32, tag="m3")
                nc.vector.tensor_tensor(out=m3[:], in0=m2[:, 0:64], in1=m2[:, 64:128],
                                        op=mybir.AluOpType.add)
                m4 = nc_pool.tile([128, GOUT], f32, tag="m4")
                nc.vector.tensor_tensor(out=m4[:], in0=m3[:], in1=b2r[:],
                                        op=mybir.AluOpType.add)
                h2m = nc_pool.tile([128, GOUT], bf16, tag="h2m")
                nc.scalar.activation(h2m[:], m4[:], AF.Copy, scale=0.125)
                pt = ps_t.tile([64, 128], bf16, tag="pst")
                nc.tensor.transpose(pt[:], h2m[:], ident[:])
                st64 = nc_pool.tile([64, 128], bf16, tag="st64")
                nc.any.tensor_copy(st64[:], pt[:])
                pm1 = ps_t.tile([64, 128], f32, tag="pst")
                nc.tensor.matmul(pm1[:], lhsT=wm1[:], rhs=st64[:], start=True,
                                 stop=True)
                hm = nc_pool.tile([64, 128], bf16, tag="hm")
                nc.scalar.activation(hm[:], pm1[:], AF.Relu, bias=bm1[:])
                pm2 = ps_small.tile([OUT_CLS, 128], f32, tag="pad")
                nc.tensor.matmul(pm2[:], lhsT=wm2[:], rhs=hm[:], start=True, stop=True)
                osb = nc_pool.tile([OUT_CLS, 128], f32, tag="osb")
                nc.vector.tensor_scalar_add(osb[:], pm2[:], bm2[:])
                nc.sync.dma_start(out_t[:, g * 128:(g + 1) * 128], osb[:])

            with tc.tile_pool(name="e2", bufs=8) as gpool2, \
                 tc.tile_pool(name="e2w", bufs=4) as nc_pool, \
                 tc.tile_pool(name="ps2n", bufs=2, space="PSUM") as ps_num, \
                 tc.tile_pool(name="ps2s", bufs=2, space="PSUM") as ps_small, \
                 tc.tile_pool(name="ps2t", bufs=2, space="PSUM") as ps_t:
                edge_phase(agbuf2[0:half_rows, 0:D2H + 8],
                           agbuf2[half_rows:NCORES * sh, 0:D2H + 8],
                           agin2, DC2, D2H, fin2,
                           lambda grp: adx2[:, grp * 8:(grp + 1) * 8],
                           gpool2, nc_pool, ps_num, ps_small, ps_t)

    nc.compile()
    return nc


def host_prep(x, edge_index, W1, a1_src, a1_dst, b1, W2, a2_src, a2_dst, b2,
              Wm1, bm1, Wm2, bm2):
    import ml_dtypes

    bf = ml_dtypes.bfloat16
    n = x.shape[0]
    n_shard = n // NCORES
    sh = ((n_shard + 127) // 128) * 128
    np_pad = NCORES * sh

    def blockdiag(a, ch):
        B = np.zeros((HEADS * ch, HEADS), np.float32)
        for hd in range(HEADS):
            B[hd * ch:(hd + 1) * ch, hd] = a[hd]
        return B

    W1 = np.asarray(W1, np.float32)
    W2 = np.asarray(W2, np.float32)
    w1aug = np.concatenate(
        [W1, W1 @ blockdiag(np.asarray(a1_src, np.float32), HID),
         W1 @ blockdiag(np.asarray(a1_dst, np.float32), HID)], axis=1).astype(bf)
    w2aug = np.concatenate(
        [W2, W2 @ blockdiag(np.asarray(a2_src, np.float32), GOUT),
         W2 @ blockdiag(np.asarray(a2_dst, np.float32), GOUT)], axis=1).astype(bf)

    xv = np.asarray(x, np.float32)
    xts = []
    for c in range(NCORES):
        xtc = np.zeros((IN_CH, sh), np.float32)
        xtc[:, :n_shard] = xv[c * n_shard:(c + 1) * n_shard].T
        xts.append(xtc.astype(bf))

    src = np.asarray(edge_index[0])
    dst = np.asarray(edge_index[1])
    spid = (src // n_shard) * sh + (src % n_shard)
    dsh = dst // n_shard
    dloc = dst % n_shard

    plan, per_core = plan_graph(spid, dsh, dloc, n_shard, sh, np_pad)

    b1res = np.broadcast_to(np.asarray(b1, np.float32)[None, :], (128, D1H)).copy()
    b2x8 = np.broadcast_to(8.0 * np.asarray(b2, np.float32)[None, :], (128, GOUT)).copy()
    iota_row = np.broadcast_to(np.arange(128, dtype=np.float32), (128, 128)).copy()

    common = {
        "iotarow": iota_row, "w1aug": w1aug,
        "w2k0": np.ascontiguousarray(w2aug[0:128]),
        "w2k1": np.ascontiguousarray(w2aug[128:256]),
        "wm1": np.asarray(Wm1, np.float32).astype(bf),
        "wm2": np.asarray(Wm2, np.float32).astype(bf),
        "b1res": b1res, "b2x8": b2x8,
        "bm1col": np.asarray(bm1, np.float32)[:, None],
        "bm2col": np.asarray(bm2, np.float32)[:, None],
    }
    in_maps = []
    for c in range(NCORES):
        idx16, dstl_in = per_core[c]
        m = dict(common)
        m["xt"] = xts[c]
        m["idx16"] = idx16
        m["dstl"] = dstl_in
        in_maps.append(m)
    return plan, in_maps, n_shard, sh


def run(inputs, trace=False):
    """Full pipeline: prep, build, run on 8 cores, assemble output."""
    import sys
    for p in ("/opt/trn_rl_repo",):
        if p not in sys.path:
            sys.path.append(p)
    from concourse.bass_utils import run_bass_kernel_spmd

    n = inputs["x"].shape[0]
    plan, in_maps, n_shard, sh = host_prep(**inputs)
    nc = build_program(n, plan)
    kw = {}
    if trace:
        import tempfile
        kw = dict(trace=True, tmpdir=tempfile.mkdtemp(prefix="gat_neff_"))
    res = run_bass_kernel_spmd(nc, in_maps, list(range(NCORES)), **kw)
    out = np.empty((n, OUT_CLS), np.float32)
    for c in range(NCORES):
        out[c * n_shard:(c + 1) * n_shard] = res.results[c]["outT"][:, :n_shard].T
    return out, res


LAST_EXEC_NS = None


def kernel(**inputs) -> np.ndarray:
    """Full-input entry point: shards/compiles/runs on 8 NeuronCores."""
    global LAST_EXEC_NS
    out, res = run(inputs)
    if getattr(res, "exec_time_ns", None) is not None:
        LAST_EXEC_NS = res.exec_time_ns
    return out


# revision 18
# speedup vs baseline: 1.1169x; 1.1169x over previous
"""GAT(2-layer, 8 heads) + MLP — full on-device Bass SPMD implementation.

Design (8 NeuronCores, graph/data parallel):
  - x is replicated (host sends xT bf16); every core computes the FULL h1
    node-transform table (redundant PE work is cheaper than exchanging it),
    writing rows [h1(256) | alpha_src(8)] bf16 into two DRAM half-tables
    (dma_gather indices are int16, so tables are limited to <32768 rows).
  - Edge phase 1 is dst-sharded: core c owns nodes [c*SH, (c+1)*SH). Edges
    (sorted by dst group, split by src half, padded to 128-edge tiles) are
    gathered with batched dma_gather; attention softmax (no max-subtraction;
    exp args are O(4)) and weighted aggregation run via per-tile
    is_equal-selection matmuls accumulated in PSUM.
  - out1^T (bf16) is AllGathered; every core then computes the full h2 table
    (layer 2), runs its edge phase 2, head-mean, and the tiny MLP, emitting
    outT [2, SH] f32 per core.
  - alpha_dst for own nodes is read from an extract-all SBUF buffer with a
    partition_id()-based dynamic slice (keeps the program SPMD-uniform).

Numerics: bf16 tables/matmuls, fp32 PSUM + softmax intermediates.
"""

import numpy as np

IN_CH = 128
HEADS = 8
HID = 32
GOUT = 64
MLP_HID = 64
OUT_CLS = 2
NEG_SLOPE = 0.2
NCORES = 8

D1H = HEADS * HID          # 256
D2H = HEADS * GOUT         # 512
DC1 = 384                  # L1 table row cols (768B)
DC2 = 640                  # L2 table row cols (1280B)


def plan_graph(src_pid, dst_shard, dst_local_in_shard, n_real_shard, sh, np_pad):
    """Build the shared tile structure + per-core edge arrays.

    Returns (plan, per_core) where plan holds compile-time lists and
    per_core holds the idx16/dstl input arrays.
    """
    groups = sh // 128
    half_rows = np_pad // 2
    counts = np.zeros((NCORES, groups, 2), np.int64)
    core_data = []
    for c in range(NCORES):
        sel = dst_shard == c
        dl = dst_local_in_shard[sel]
        sp = src_pid[sel]
        half = (sp >= half_rows).astype(np.int64)
        sloc = sp - half * half_rows
        grp = dl >> 7
        dstl = dl & 127
        order = np.lexsort((half, grp))
        grp, half, sloc, dstl = grp[order], half[order], sloc[order], dstl[order]
        key = grp * 2 + half
        counts[c] = np.bincount(key, minlength=groups * 2).reshape(groups, 2)
        core_data.append((key, sloc, dstl))

    tiles = np.ceil(counts.max(axis=0) / 128.0).astype(np.int64)  # [groups, 2]

    # stream order: per quad of 4 groups, all A tiles then all B tiles
    quads = [list(range(q, min(q + 4, groups))) for q in range(0, groups, 4)]
    tile_list = []          # (g, h) per tile
    chunks = []             # (half, start_tile, ntiles) in stream order
    quad_chunks = []        # per quad: list of chunks
    group_tiles = [[None, None] for _ in range(groups)]  # (start_tile, n) per half
    for quad in quads:
        qc = []
        for h in (0, 1):
            start = len(tile_list)
            for g in quad:
                group_tiles[g][h] = (len(tile_list), int(tiles[g][h]))
                tile_list += [(g, h)] * int(tiles[g][h])
            if len(tile_list) > start:
                chunks.append((h, start, len(tile_list) - start))
                qc.append((h, start, len(tile_list) - start))
        quad_chunks.append(qc)
    nt = len(tile_list)

    first_tile = np.zeros((groups, 2), np.int64)
    for g in range(groups):
        for h in (0, 1):
            first_tile[g, h] = group_tiles[g][h][0]

    per_core = []
    for c in range(NCORES):
        key, sloc, dstl = core_data[c]
        # rank within (g,h) segment
        seg_start = np.zeros(groups * 2, np.int64)
        cnt = counts[c].reshape(-1)
        np.cumsum(cnt[:-1], out=seg_start[1:])
        rank = np.arange(len(key)) - seg_start[key]
        slot = first_tile.reshape(-1)[key] * 128 + rank
        idx_flat = np.zeros(nt * 128, np.int16)
        dstl_flat = np.full(nt * 128, -1.0, np.float32)
        idx_flat[slot] = sloc.astype(np.int16)
        dstl_flat[slot] = dstl.astype(np.float32)
        idx16 = np.tile(idx_flat.reshape(-1, 16).T, (8, 1))  # [128, nt*8]
        dstl_in = dstl_flat.reshape(nt, 128).T.copy()        # [128, nt]
        per_core.append((idx16, dstl_in))

    plan = dict(groups=groups, quads=quads, chunks=chunks,
                group_tiles=group_tiles, nt=nt, half_rows=half_rows,
                tile_list=tile_list, quad_chunks=quad_chunks)
    return plan, per_core


def build_program(n_real, plan, use_prelu=True):
    import sys
    for p in ("/opt/trn_rl_repo",):
        if p not in sys.path:
            sys.path.append(p)
    from concourse import bacc, mybir, bass
    import concourse.tile as tile
    from concourse.masks import make_identity
    from concourse.bass import ds, ts
    import concourse.bass as _B
    import inspect as _inspect
    import textwrap as _tw

    # dma_gather with the 256B elem_size restriction lifted (transpose-only
    # constraint; non-transpose path handles arbitrary elem sizes).
    _src = _inspect.getsource(_B.BassGpSimd.dma_gather)
    _src = _src.replace(
        """        assert (
            elem_size_bytes > 0 and elem_size_bytes % 256 == 0
        )  # transpose restriction""", "        assert elem_size_bytes > 0")
    _ns = dict(_B.__dict__)
    exec(_tw.dedent(_src), _ns)
    dma_gather_raw = _ns["dma_gather"]

    f32 = mybir.dt.float32
    bf16 = mybir.dt.bfloat16
    i16 = mybir.dt.int16
    AF = mybir.ActivationFunctionType

    groups = plan["groups"]
    sh = groups * 128
    np_pad = NCORES * sh
    half_rows = plan["half_rows"]
    half_tiles = half_rows // 128
    n_tiles = np_pad // 128
    nt = plan["nt"]
    own_stride = groups * HEADS  # per-core alpha_d columns

    nc = bacc.Bacc("TRN2", target_bir_lowering=False, debug=False,
                   num_devices=NCORES, num_swdge_queues=4)

    # ---- I/O ----
    xt_in = nc.dram_tensor("xt", [IN_CH, sh], bf16, kind="ExternalInput")
    idx_in = nc.dram_tensor("idx16", [128, nt * 8], i16, kind="ExternalInput")
    dstl_in = nc.dram_tensor("dstl", [128, nt], f32, kind="ExternalInput")
    iota_in = nc.dram_tensor("iotarow", [128, 128], f32, kind="ExternalInput")
    w1_in = nc.dram_tensor("w1aug", [IN_CH, 272], bf16, kind="ExternalInput")
    w2a_in = nc.dram_tensor("w2k0", [128, 528], bf16, kind="ExternalInput")
    w2b_in = nc.dram_tensor("w2k1", [128, 528], bf16, kind="ExternalInput")
    wm1_in = nc.dram_tensor("wm1", [MLP_HID, MLP_HID], bf16, kind="ExternalInput")
    wm2_in = nc.dram_tensor("wm2", [MLP_HID, OUT_CLS], bf16, kind="ExternalInput")
    b1_in = nc.dram_tensor("b1res", [128, D1H], f32, kind="ExternalInput")
    b2_in = nc.dram_tensor("b2x8", [128, GOUT], f32, kind="ExternalInput")
    bm1_in = nc.dram_tensor("bm1col", [MLP_HID, 1], f32, kind="ExternalInput")
    bm2_in = nc.dram_tensor("bm2col", [OUT_CLS, 1], f32, kind="ExternalInput")
    out_t = nc.dram_tensor("outT", [OUT_CLS, sh], f32, kind="ExternalOutput")

    # ---- internal DRAM ----
    agin1 = nc.dram_tensor("agin1", [sh, DC1], bf16)
    agbuf1 = nc.dram_tensor("agbuf1", [NCORES * sh, DC1], bf16,
                            addr_space="Shared")
    agin2 = nc.dram_tensor("agin2", [sh, DC2], bf16)
    agbuf2 = nc.dram_tensor("agbuf2", [NCORES * sh, DC2], bf16,
                            addr_space="Shared")

    with tile.TileContext(nc) as tc:
        with tc.tile_pool(name="res", bufs=1) as res:
            # residents
            iota = res.tile([128, 128], f32, tag="iota")
            nc.sync.dma_start(iota[:], iota_in[:])
            ident = res.tile([128, 128], bf16, tag="ident")
            make_identity(nc, ident[:])
            idxt = res.tile([128, nt * 8], i16, tag="idx")
            nc.sync.dma_start(idxt[:], idx_in[:])
            dstl = res.tile([128, nt], f32, tag="dstl")
            nc.sync.dma_start(dstl[:], dstl_in[:])
            b1r = res.tile([128, D1H], f32, tag="b1r")
            nc.sync.dma_start(b1r[:], b1_in[:])
            b2r = res.tile([128, GOUT], f32, tag="b2r")
            nc.sync.dma_start(b2r[:], b2_in[:])
            wm1 = res.tile([MLP_HID, MLP_HID], bf16, tag="wm1")
            nc.sync.dma_start(wm1[:], wm1_in[:])
            wm2 = res.tile([MLP_HID, OUT_CLS], bf16, tag="wm2")
            nc.sync.dma_start(wm2[:], wm2_in[:])
            bm1 = res.tile([MLP_HID, 1], f32, tag="bm1")
            nc.sync.dma_start(bm1[:], bm1_in[:])
            bm2 = res.tile([OUT_CLS, 1], f32, tag="bm2")
            nc.sync.dma_start(bm2[:], bm2_in[:])
            adx = res.tile([128, groups * HEADS], bf16, tag="adx")
            adx2 = res.tile([128, groups * HEADS], bf16, tag="adx2")

            # ---- generic edge phase ----
            def edge_phase(tbl_a, tbl_b, agin, dcols, dh, finalize, ad_ap, gpool,
                           wpool, ps_num, ps_small, ps_t):
                tile_group = plan["tile_list"]
                oct_art = {}
                qn = 0
                for qi, quad in enumerate(plan["quads"]):
                    for (h, cstart, ntl) in plan["quad_chunks"][qi]:
                        tbl = tbl_b if h else tbl_a
                        for k0 in range(0, ntl, 8):
                            kn = min(8, ntl - k0)
                            gt0 = cstart + k0
                            el = dh + 8
                            g_t = gpool.tile([128, 8, el], bf16, tag="g")
                            dma_gather_raw(
                                nc.gpsimd,
                                out_ap=g_t[:, 0:kn, :], in_ap=tbl,
                                idxs_ap=idxt[:, gt0 * 8:(gt0 + kn) * 8],
                                num_idxs=kn * 128, num_idxs_reg=kn * 128,
                                elem_size=el, elem_step=dcols, queue_num=qn % 4)
                            qn += 1
                            sal = wpool.tile([128, 8, 128], bf16, tag="S")
                            nc.vector.tensor_tensor(
                                out=sal[:, 0:kn, :],
                                in0=dstl[:, gt0:gt0 + kn].unsqueeze(2).to_broadcast(
                                    [128, kn, 128]),
                                in1=iota[:].unsqueeze(1).to_broadcast([128, kn, 128]),
                                op=mybir.AluOpType.is_equal)
                            pad_all = ps_small.tile([128, 64], f32, tag="pad")
                            for j in range(kn):
                                grp = tile_group[gt0 + j][0]
                                pst = ps_t.tile([128, 128], bf16, tag="pst")
                                nc.tensor.transpose(pst[:], sal[:, j, :], ident[:])
                                st = wpool.tile([128, 128], bf16, tag="st")
                                nc.any.tensor_copy(st[:], pst[:])
                                nc.tensor.matmul(
                                    pad_all[:, j * 8:(j + 1) * 8], lhsT=st[:],
                                    rhs=ad_ap(grp), start=True, stop=True)
                            esum = wpool.tile([128, 64], f32, tag="esum")
                            nc.vector.tensor_tensor(
                                out=esum[:, 0:kn * 8].rearrange(
                                    "p (a b) -> p a b", a=kn),
                                in0=g_t[:, 0:kn, dh:dh + 8],
                                in1=pad_all[:, 0:kn * 8].rearrange(
                                    "p (a b) -> p a b", a=kn),
                                op=mybir.AluOpType.add)
                            elr = wpool.tile([128, 64], f32, tag="elr")
                            if use_prelu:
                                nc.scalar.activation(elr[:, 0:kn * 8],
                                                     esum[:, 0:kn * 8], AF.Prelu,
                                                     alpha=NEG_SLOPE)
                            else:
                                esc = wpool.tile([128, 64], f32, tag="esc")
                                nc.vector.tensor_scalar_mul(
                                    esc[:, 0:kn * 8], esum[:, 0:kn * 8], NEG_SLOPE)
                                nc.vector.tensor_tensor(
                                    out=elr[:, 0:kn * 8], in0=esum[:, 0:kn * 8],
                                    in1=esc[:, 0:kn * 8], op=mybir.AluOpType.max)
                            ex = wpool.tile([128, 64], bf16, tag="ex")
                            nc.scalar.activation(ex[:, 0:kn * 8], elr[:, 0:kn * 8],
                                                 AF.Exp)
                            ch = dh // HEADS
                            wg = wpool.tile([128, 8, dh], bf16, tag="wg")
                            nc.vector.tensor_tensor(
                                out=wg[:, 0:kn, :].rearrange(
                                    "p a (h c) -> p a h c", h=HEADS),
                                in0=g_t[:, 0:kn, 0:dh].rearrange(
                                    "p a (h c) -> p a h c", h=HEADS),
                                in1=ex[:, 0:kn * 8].rearrange(
                                    "p (a h) -> p a h", a=kn).unsqueeze(3)
                                    .to_broadcast([128, kn, HEADS, ch]),
                                op=mybir.AluOpType.mult)
                            for j in range(kn):
                                oct_art[gt0 + j] = (sal, ex, wg, j)
                    for g in quad:
                        pnum = ps_num.tile([128, dh], f32, tag="num")
                        pden = ps_small.tile([128, HEADS], f32, tag="den")
                        # self-loop tile: direct local read from agin (own shard)
                        gs = wpool.tile([128, dh + 8], bf16, tag="gs")
                        nc.sync.dma_start(gs[:], agin[g * 128:(g + 1) * 128, 0:dh + 8])
                        esums = wpool.tile([128, HEADS], f32, tag="esums")
                        nc.vector.tensor_tensor(out=esums[:], in0=gs[:, dh:dh + 8],
                                                in1=ad_ap(g), op=mybir.AluOpType.add)
                        elrs = wpool.tile([128, HEADS], f32, tag="elrs")
                        if use_prelu:
                            nc.scalar.activation(elrs[:], esums[:], AF.Prelu,
                                                 alpha=NEG_SLOPE)
                        else:
                            escs = wpool.tile([128, HEADS], f32, tag="escs")
                            nc.vector.tensor_scalar_mul(escs[:], esums[:], NEG_SLOPE)
                            nc.vector.tensor_tensor(out=elrs[:], in0=esums[:],
                                                    in1=escs[:],
                                                    op=mybir.AluOpType.max)
                        exs = wpool.tile([128, HEADS], bf16, tag="exs")
                        nc.scalar.activation(exs[:], elrs[:], AF.Exp)
                        ch = dh // HEADS
                        wgs = wpool.tile([128, dh], bf16, tag="wgs")
                        nc.vector.tensor_tensor(
                            out=wgs[:].rearrange("p (h c) -> p h c", h=HEADS),
                            in0=gs[:, 0:dh].rearrange("p (h c) -> p h c", h=HEADS),
                            in1=exs[:].unsqueeze(2).to_broadcast([128, HEADS, ch]),
                            op=mybir.AluOpType.mult)
                        tl = []
                        for h in (0, 1):
                            st_t, n_t = plan["group_tiles"][g][h]
                            tl += [st_t + i for i in range(n_t)]
                        nc.tensor.matmul(pnum[:], lhsT=ident[:], rhs=wgs[:],
                                         start=True, stop=(len(tl) == 0))
                        nc.tensor.matmul(pden[:], lhsT=ident[:], rhs=exs[:],
                                         start=True, stop=(len(tl) == 0))
                        for j, t in enumerate(tl):
                            sal, ex, wg, jj = oct_art.pop(t)
                            nc.tensor.matmul(pnum[:], lhsT=sal[:, jj, :],
                                             rhs=wg[:, jj, :],
                                             start=False, stop=(j == len(tl) - 1))
                            nc.tensor.matmul(pden[:], lhsT=sal[:, jj, :],
                                             rhs=ex[:, jj * 8:(jj + 1) * 8],
                                             start=False, stop=(j == len(tl) - 1))
                        finalize(g, pnum, pden, ps_t=ps_t, ps_num=ps_num,
                                 ps_small=ps_small)

            # ---- phase 1: mm1' (own shard) -> agin1; AG1 -> full L1 table ----
            with tc.tile_pool(name="mm1", bufs=1) as mm1p, \
                 tc.tile_pool(name="stg1", bufs=3) as stg1, \
                 tc.tile_pool(name="ps_mm1", bufs=3, space="PSUM") as ps_mm1:
                xt = mm1p.tile([IN_CH, sh], bf16, tag="xt")
                nc.sync.dma_start(xt[:], xt_in[:])
                w1 = mm1p.tile([IN_CH, 272], bf16, tag="w1")
                nc.sync.dma_start(w1[:], w1_in[:])
                bat = 4
                for t0 in range(0, groups, bat):
                    nb = min(bat, groups - t0)
                    stg = stg1.tile([128, bat, 264], bf16, tag="stg")
                    for j in range(nb):
                        t = t0 + j
                        pm = ps_mm1.tile([128, 272], f32, tag="pmm")
                        nc.tensor.matmul(pm[:], lhsT=xt[:, t * 128:(t + 1) * 128],
                                         rhs=w1[:], start=True, stop=True)
                        nc.any.tensor_copy(stg[:, j, 0:264], pm[:, 0:264])
                        nc.any.tensor_copy(adx[:, t * 8:(t + 1) * 8], pm[:, 264:272])
                    dst_ap = agin1[t0 * 128:(t0 + nb) * 128, 0:264].rearrange(
                        "(b r) c -> r b c", b=nb)
                    nc.sync.dma_start(dst_ap, stg[:, 0:nb, :])

            nc.gpsimd.collective_compute(
                "AllGather", mybir.AluOpType.bypass,
                replica_groups=[list(range(NCORES))],
                ins=[agin1[:]], outs=[agbuf1[:]])

            # ---- phase 2: edge phase 1 -> o1sb (transposed, SBUF) ----
            with tc.tile_pool(name="o1p", bufs=1) as o1p:
                o1sb = o1p.tile([128, 2, sh], bf16, tag="o1sb")

                def fin1(g, pnum, pden, ps_t=None, ps_num=None, ps_small=None):
                    den = nc_pool.tile([128, HEADS], f32, tag="denc")
                    nc.any.tensor_copy(den[:], pden[:])
                    rec = nc_pool.tile([128, HEADS], f32, tag="rec")
                    nc.vector.reciprocal(rec[:], den[:])
                    v = nc_pool.tile([128, D1H], f32, tag="v1")
                    nc.vector.tensor_tensor(
                        out=v[:].rearrange("p (h c) -> p h c", h=HEADS),
                        in0=pnum[:].rearrange("p (h c) -> p h c", h=HEADS),
                        in1=rec[:].unsqueeze(2).to_broadcast([128, HEADS, HID]),
                        op=mybir.AluOpType.mult)
                    va = nc_pool.tile([128, D1H], f32, tag="va1")
                    nc.vector.tensor_tensor(out=va[:], in0=v[:], in1=b1r[:],
                                            op=mybir.AluOpType.add)
                    h1r = nc_pool.tile([128, D1H], bf16, tag="h1r")
                    nc.scalar.activation(h1r[:], va[:], AF.Relu)
                    for i in range(2):
                        pt = ps_t.tile([128, 128], bf16, tag="pst")
                        nc.tensor.transpose(pt[:], h1r[:, i * 128:(i + 1) * 128],
                                            ident[:])
                        nc.any.tensor_copy(o1sb[:, i, g * 128:(g + 1) * 128], pt[:])

                with tc.tile_pool(name="e1", bufs=8) as gpool1, \
                     tc.tile_pool(name="e1w", bufs=6) as nc_pool, \
                     tc.tile_pool(name="ps1n", bufs=2, space="PSUM") as ps_num, \
                     tc.tile_pool(name="ps1s", bufs=2, space="PSUM") as ps_small, \
                     tc.tile_pool(name="ps1t", bufs=2, space="PSUM") as ps_t:
                    edge_phase(agbuf1[0:half_rows, 0:D1H + 8],
                               agbuf1[half_rows:NCORES * sh, 0:D1H + 8],
                               agin1, DC1, D1H, fin1,
                               lambda grp: adx[:, grp * 8:(grp + 1) * 8],
                               gpool1, nc_pool, ps_num, ps_small, ps_t)

                # ---- phase 3: mm2' (own shard only) -> agin2 ----
                with tc.tile_pool(name="mm2", bufs=1) as mm2p, \
                     tc.tile_pool(name="stg2", bufs=3) as stg2, \
                     tc.tile_pool(name="ps_mm2", bufs=2, space="PSUM") as ps_mm2:
                    w2sb = []
                    for kb in range(2):
                        w2t = mm2p.tile([128, 528], bf16, tag=f"w2_{kb}")
                        w2sb.append(w2t)
                    nc.sync.dma_start(w2sb[0][:], w2a_in[:])
                    nc.sync.dma_start(w2sb[1][:], w2b_in[:])
                    bat = 4
                    for lt0 in range(0, groups, bat):
                        nb = min(bat, groups - lt0)
                        stg = stg2.tile([128, bat, 520], bf16, tag="stg")
                        for j in range(nb):
                            lt = lt0 + j
                            pm = ps_mm2.tile([128, 512], f32, tag="num")
                            pmb = ps_mm2.tile([128, 16], f32, tag="pad")
                            for kb in range(2):
                                nc.tensor.matmul(
                                    pm[:], lhsT=o1sb[:, kb, lt * 128:(lt + 1) * 128],
                                    rhs=w2sb[kb][:, 0:512],
                                    start=(kb == 0), stop=(kb == 1))
                            for kb in range(2):
                                nc.tensor.matmul(
                                    pmb[:], lhsT=o1sb[:, kb, lt * 128:(lt + 1) * 128],
                                    rhs=w2sb[kb][:, 512:528],
                                    start=(kb == 0), stop=(kb == 1))
                            nc.any.tensor_copy(stg[:, j, 0:512], pm[:])
                            nc.any.tensor_copy(stg[:, j, 512:520], pmb[:, 0:8])
                            nc.any.tensor_copy(adx2[:, lt * 8:(lt + 1) * 8],
                                               pmb[:, 8:16])
                        dst_ap = agin2[lt0 * 128:(lt0 + nb) * 128, 0:520].rearrange(
                            "(b r) c -> r b c", b=nb)
                        nc.sync.dma_start(dst_ap, stg[:, 0:nb, :])

            # ---- phase 4: AllGather shard tables -> full L2 gather table ----
            nc.gpsimd.collective_compute(
                "AllGather", mybir.AluOpType.bypass,
                replica_groups=[list(range(NCORES))],
                ins=[agin2[:]], outs=[agbuf2[:]])

            # ---- phase 5: edge phase 2 + MLP ----
            def fin2(g, pnum, pden, ps_t=None, ps_num=None, ps_small=None):
                den = nc_pool.tile([128, HEADS], f32, tag="denc")
                nc.any.tensor_copy(den[:], pden[:])
                rec = nc_pool.tile([128, HEADS], f32, tag="rec")
                nc.vector.reciprocal(rec[:], den[:])
                v = nc_pool.tile([128, D2H], f32, tag="v2")
                nc.vector.tensor_tensor(
                    out=v[:].rearrange("p (h c) -> p h c", h=HEADS),
                    in0=pnum[:].rearrange("p (h c) -> p h c", h=HEADS),
                    in1=rec[:].unsqueeze(2).to_broadcast([128, HEADS, GOUT]),
                    op=mybir.AluOpType.mult)
                m1 = nc_pool.tile([128, 256], f32, tag="m1")
                nc.vector.tensor_tensor(out=m1[:], in0=v[:, 0:256], in1=v[:, 256:512],
                                        op=mybir.AluOpType.add)
                m2 = nc_pool.tile([128, 128], f32, tag="m2")
                nc.vector.tensor_tensor(out=m2[:], in0=m1[:, 0:128], in1=m1[:, 128:256],
                                        op=mybir.AluOpType.add)
                m3 = nc_pool.tile([128, GOUT], f32, tag="m3")
                nc.vector.tensor_tensor(out=m3[:], in0=m2[:, 0:64], in1=m2[:, 64:128],
                                        op=mybir.AluOpType.add)
                m4 = nc_pool.tile([128, GOUT], f32, tag="m4")
                nc.vector.tensor_tensor(out=m4[:], in0=m3[:], in1=b2r[:],
                                        op=mybir.AluOpType.add)
                h2m = nc_pool.tile([128, GOUT], bf16, tag="h2m")
                nc.scalar.activation(h2m[:], m4[:], AF.Copy, scale=0.125)
                pt = ps_t.tile([64, 128], bf16, tag="pst")
                nc.tensor.transpose(pt[:], h2m[:], ident[:])
                st64 = nc_pool.tile([64, 128], bf16, tag="st64")
                nc.any.tensor_copy(st64[:], pt[:])
                pm1 = ps_t.tile([64, 128], f32, tag="pst")
                nc.tensor.matmul(pm1[:], lhsT=wm1[:], rhs=st64[:], start=True,
                                 stop=True)
                hm = nc_pool.tile([64, 128], bf16, tag="hm")
                nc.scalar.activation(hm[:], pm1[:], AF.Relu, bias=bm1[:])
                pm2 = ps_small.tile([OUT_CLS, 128], f32, tag="pad")
                nc.tensor.matmul(pm2[:], lhsT=wm2[:], rhs=hm[:], start=True, stop=True)
                osb = nc_pool.tile([OUT_CLS, 128], f32, tag="osb")
                nc.vector.tensor_scalar_add(osb[:], pm2[:], bm2[:])
                nc.sync.dma_start(out_t[:, g * 128:(g + 1) * 128], osb[:])

            with tc.tile_pool(name="e2", bufs=8) as gpool2, \
                 tc.tile_pool(name="e2w", bufs=6) as nc_pool, \
                 tc.tile_pool(name="ps2n", bufs=2, space="PSUM") as ps_num, \
                 tc.tile_pool(name="ps2s", bufs=2, space="PSUM") as ps_small, \
                 tc.tile_pool(name="ps2t", bufs=2, space="PSUM") as ps_t:
                edge_phase(agbuf2[0:half_rows, 0:D2H + 8],
                           agbuf2[half_rows:NCORES * sh, 0:D2H + 8],
                           agin2, DC2, D2H, fin2,
                           lambda grp: adx2[:, grp * 8:(grp + 1) * 8],
                           gpool2, nc_pool, ps_num, ps_small, ps_t)

    nc.compile()
    return nc


def host_prep(x, edge_index, W1, a1_src, a1_dst, b1, W2, a2_src, a2_dst, b2,
              Wm1, bm1, Wm2, bm2):
    import ml_dtypes

    bf = ml_dtypes.bfloat16
    n = x.shape[0]
    n_shard = n // NCORES
    sh = ((n_shard + 127) // 128) * 128
    np_pad = NCORES * sh

    def blockdiag(a, ch):
        B = np.zeros((HEADS * ch, HEADS), np.float32)
        for hd in range(HEADS):
            B[hd * ch:(hd + 1) * ch, hd] = a[hd]
        return B

    W1 = np.asarray(W1, np.float32)
    W2 = np.asarray(W2, np.float32)
    w1aug = np.concatenate(
        [W1, W1 @ blockdiag(np.asarray(a1_src, np.float32), HID),
         W1 @ blockdiag(np.asarray(a1_dst, np.float32), HID)], axis=1).astype(bf)
    w2aug = np.concatenate(
        [W2, W2 @ blockdiag(np.asarray(a2_src, np.float32), GOUT),
         W2 @ blockdiag(np.asarray(a2_dst, np.float32), GOUT)], axis=1).astype(bf)

    xv = np.asarray(x, np.float32)
    xts = []
    for c in range(NCORES):
        xtc = np.zeros((IN_CH, sh), np.float32)
        xtc[:, :n_shard] = xv[c * n_shard:(c + 1) * n_shard].T
        xts.append(xtc.astype(bf))

    src = np.asarray(edge_index[0])
    dst = np.asarray(edge_index[1])
    spid = (src // n_shard) * sh + (src % n_shard)
    dsh = dst // n_shard
    dloc = dst % n_shard

    plan, per_core = plan_graph(spid, dsh, dloc, n_shard, sh, np_pad)

    b1res = np.broadcast_to(np.asarray(b1, np.float32)[None, :], (128, D1H)).copy()
    b2x8 = np.broadcast_to(8.0 * np.asarray(b2, np.float32)[None, :], (128, GOUT)).copy()
    iota_row = np.broadcast_to(np.arange(128, dtype=np.float32), (128, 128)).copy()

    common = {
        "iotarow": iota_row, "w1aug": w1aug,
        "w2k0": np.ascontiguousarray(w2aug[0:128]),
        "w2k1": np.ascontiguousarray(w2aug[128:256]),
        "wm1": np.asarray(Wm1, np.float32).astype(bf),
        "wm2": np.asarray(Wm2, np.float32).astype(bf),
        "b1res": b1res, "b2x8": b2x8,
        "bm1col": np.asarray(bm1, np.float32)[:, None],
        "bm2col": np.asarray(bm2, np.float32)[:, None],
    }
    in_maps = []
    for c in range(NCORES):
        idx16, dstl_in = per_core[c]
        m = dict(common)
        m["xt"] = xts[c]
        m["idx16"] = idx16
        m["dstl"] = dstl_in
        in_maps.append(m)
    return plan, in_maps, n_shard, sh


def run(inputs, trace=False):
    """Full pipeline: prep, build, run on 8 cores, assemble output."""
    import sys
    for p in ("/opt/trn_rl_repo",):
        if p not in sys.path:
            sys.path.append(p)
    from concourse.bass_utils import run_bass_kernel_spmd

    n = inputs["x"].shape[0]
    plan, in_maps, n_shard, sh = host_prep(**inputs)
    nc = build_program(n, plan)
    kw = {}
    if trace:
        import tempfile
        kw = dict(trace=True, tmpdir=tempfile.mkdtemp(prefix="gat_neff_"))
    res = run_bass_kernel_spmd(nc, in_maps, list(range(NCORES)), **kw)
    out = np.empty((n, OUT_CLS), np.float32)
    for c in range(NCORES):
        out[c * n_shard:(c + 1) * n_shard] = res.results[c]["outT"][:, :n_shard].T
    return out, res


LAST_EXEC_NS = None


def kernel(**inputs) -> np.ndarray:
    """Full-input entry point: shards/compiles/runs on 8 NeuronCores."""
    global LAST_EXEC_NS
    out, res = run(inputs)
    if getattr(res, "exec_time_ns", None) is not None:
        LAST_EXEC_NS = res.exec_time_ns
    return out
